# revision 5
# baseline (speedup 1.0000x reference)
"""ConcatCritic MLP over the B^2 pair grid, Trainium2 Bass/Tile kernel.

out[i, j] = softplus(f(x[i], y[j])) where f is a 4-layer MLP on
concat(x, y). Reference pair grid: pairs[a*n+b] = concat(x[b], y[a]),
scores.reshape(n,n).T -> out.

Key factorization: layer 1 is linear in the concat, so
  h1[a,b] = relu(x[b] @ W1top + y[a] @ W1bot + b1)
with W1top = W1[:128], W1bot = W1[128:]. The [B^2, 256] layer-1 matmul
collapses into two tiny matmuls plus a per-partition broadcast add.

Layout: activations kept transposed [features, batch] so every layer's
matmul (weights stationary as lhsT [K, M]) writes the next layer's rhs
directly: out[m=feat, n=j] = sum_k W[k, m] * hT[k, j].

Sharding: core c owns y rows [c*64, (c+1)*64); computes block
V_c[il, j] = f(x[j], y[c*64+il]) of shape [64, 512]. Host gathers
V = concat(V_c) and returns V.T.

Fast path (all biases zero, which is what setup_inputs produces):
fp16 weights + activations. PE matmul stays 1 cycle/row (same as
fp32r) but LDWEIGHTS halves, DVE L1 runs in 4x perf mode, input DMA
halves, and the drain path avoids the mid-run Ln that forced act-table
reloads in the fp32r variant (4 loads -> 2). Measured fp16 rel err vs
fp64 reference ~4e-4 (budget 2e-2).

Generic path (any nonzero bias): the original fp32r kernel, unchanged.

Self-contained: hardcodes shapes; imports concourse from the system repo.
"""

import os
import sys

import numpy as np
import ml_dtypes


def _import_concourse():
    try:
        import concourse  # noqa: F401
        return
    except ImportError:
        pass
    for p in ("/opt/trn_rl_repo", "/root/.axon_site/_ro/trn_rl_repo"):
        if os.path.isdir(p) and p not in sys.path:
            sys.path.insert(0, p)
    import concourse  # noqa: F401


_import_concourse()

import concourse.bacc as bacc  # noqa: E402
import concourse.tile as tile  # noqa: E402
from concourse import mybir  # noqa: E402
from concourse.bass_utils import run_bass_kernel_spmd  # noqa: E402

B = 512          # batch (pair-grid side)
D = 128          # input dim per tensor
H = 256          # hidden dim
NCORES = 8
RB = B // NCORES  # 64 y-rows per core
F32 = mybir.dt.float32
F16 = mybir.dt.float16

# float32r: fp32 bits in memory, single-pass reduced-precision multiply on
# the PE at 1 cycle/row (vs 4 for full fp32). Used by the generic path.
PRE_DT = mybir.dt.float32r
MM_DT = mybir.dt.float32r


def _emit_f16(tc, nc, d, out_d):
    """Fast path: all biases zero, fp16 weights/activations.

    Per step the PE runs 10 matmuls (~216ns each incl. hidden fp16
    LDWEIGHTS) in order L2MM(t) | L3MM(t-1) | L4MM(t-2), so every PE
    instruction depends only on epilogue work from a previous step.
    ACT: one merged [128,1024] relu over both L2 psum banks + L3epi mc1
    (~1.8us). DVE: L3epi mc0 early, then 2x L1 in 4x perf mode
    (~1.4us). exp() batches 4 steps via one [97,512] pass over the
    shared L4 psum bank (matmuls write partition bases 0/32/64/96);
    a strided DMA scatters the 4 valid rows into score. Single Ln at
    the end: exactly two act-table loads for the whole kernel.
    PSUM banks: l2 [128,1024] bufs=2 (4) + l3 [128,1024] bufs=1 (2) +
    l4 [97,512] bufs=2 (2) = 8 exactly.
    """
    AF = mybir.ActivationFunctionType
    OP = mybir.AluOpType
    from contextlib import ExitStack

    with ExitStack() as ctx:
        const = ctx.enter_context(tc.tile_pool(name="const", bufs=1))
        hpool = ctx.enter_context(tc.tile_pool(name="h", bufs=2))
        psum = ctx.enter_context(tc.tile_pool(name="psum", bufs=1, space="PSUM"))

        def load(name, shape, src_ap=None, eng=None):
            t = const.tile(list(shape), F16, tag=name, name=name + "_s")
            src = src_ap if src_ap is not None else d[name][:]
            (eng or nc.sync).dma_start(out=t[:], in_=src)
            return t

        # Load order = first-use order. The per-DMA trigger costs ~600ns of
        # engine-queue time, so split issue across the idle gpsimd queue
        # (first-needed tensors) and sync (weights needed a few steps in).
        yT = load("yT", (D, RB), eng=nc.gpsimd)
        w1b = load("W1b", (D, H), eng=nc.gpsimd)
        xT = load("xT", (D, B), eng=nc.gpsimd)
        w1t = load("W1t", (D, H), eng=nc.gpsimd)
        w2 = [load(f"W2_{k}", (128, H), d["W2"][k * 128:(k + 1) * 128, :]) for k in range(2)]
        w3 = [load(f"W3_{k}", (128, H), d["W3"][k * 128:(k + 1) * 128, :]) for k in range(2)]
        w4 = [load(f"W4_{k}", (128, 1), d["W4"][k * 128:(k + 1) * 128, :]) for k in range(2)]

        # ---- preamble: xa[oc] = (x @ W1top)^T, yb[oc] = (y_slice @ W1bot)^T
        # Preamble psum tiles borrow the main-loop l2/l3 tags (no extra
        # banks). yb first (tiny copies), xa epilogue on ACT so the DVE can
        # start L1(0) as soon as xa lands.
        yb = []
        pl3 = psum.tile([128, 2 * B], F32, tag="l3", name="pyb", bufs=1)
        for oc in range(2):
            ms = slice(oc * 128, (oc + 1) * 128)
            nc.tensor.matmul(pl3[:, oc * B:oc * B + RB], lhsT=w1b[:, ms], rhs=yT[:],
                             start=True, stop=True)
            ybt = const.tile([128, RB], F32, tag=f"yb{oc}", name=f"yb{oc}")
            nc.vector.tensor_copy(ybt[:], pl3[:, oc * B:oc * B + RB])
            yb.append(ybt)
        xa = []
        pl2 = psum.tile([128, 2 * B], F32, tag="l2", name="pxa", bufs=2)
        for oc in range(2):
            ms = slice(oc * 128, (oc + 1) * 128)
            nc.tensor.matmul(pl2[:, oc * B:(oc + 1) * B], lhsT=w1t[:, ms], rhs=xT[:],
                             start=True, stop=True)
            xat = const.tile([128, B], F16, tag=f"xa{oc}", name=f"xa{oc}")
            nc.scalar.activation(xat[:], pl2[:, oc * B:(oc + 1) * B], AF.Copy, bias=0.0)
            xa.append(xat)

        score = const.tile([RB, B], F32, tag="score", name="score")

        h1s, h2s, h3s = {}, {}, {}
        p4x = [None]  # current 4-step L4 psum tile

        def emit_l1(i):
            # DVE, 4x perf mode: fp16 in/out, per-partition fp32 scalar yb.
            for oc in range(2):
                t = hpool.tile([128, B], F16, tag=f"h1_{oc}", name=f"h1_{oc}_{i}", bufs=4)
                nc.vector.tensor_scalar(
                    t[:], xa[oc][:], yb[oc][:, i:i + 1], 0.0, OP.add, OP.max
                )
                h1s[i, oc] = t

        def emit_l2mm(i):
            p = psum.tile([128, 2 * B], F32, tag="l2", name=f"p2_{i}", bufs=2)
            for mc in range(2):
                ms = slice(mc * 128, (mc + 1) * 128)
                for kc in range(2):
                    nc.tensor.matmul(
                        p[:, mc * B:(mc + 1) * B], lhsT=w2[kc][:, ms], rhs=h1s[i, kc][:],
                        start=(kc == 0), stop=(kc == 1),
                    )
            h2s[i] = p

        def emit_l2epi(i):
            # One merged [128,1024] relu over both L2 psum banks (ACT).
            p = h2s[i]
            t = hpool.tile([128, 2 * B], F16, tag="h2", name=f"h2_{i}", bufs=3)
            nc.scalar.activation(t[:], p[:], AF.Relu)
            h2s[i] = t

        def emit_l3mm(i):
            p = psum.tile([128, 2 * B], F32, tag="l3", name=f"p3_{i}", bufs=1)
            for mc in range(2):
                ms = slice(mc * 128, (mc + 1) * 128)
                for kc in range(2):
                    nc.tensor.matmul(
                        p[:, mc * B:(mc + 1) * B], lhsT=w3[kc][:, ms],
                        rhs=h2s[i][:, kc * B:(kc + 1) * B],
                        start=(kc == 0), stop=(kc == 1),
                    )
            h3s[i] = p

        def emit_l3epi(i):
            # mc0 on DVE, mc1 on ACT: balances both engines ~0.6-0.7us under
            # the PE step time.
            p = h3s[i]
            ta = hpool.tile([128, B], F16, tag="h3_0", name=f"h3_0_{i}", bufs=3)
            nc.vector.tensor_scalar_max(ta[:], p[:, 0:B], 0.0)
            tb = hpool.tile([128, B], F16, tag="h3_1", name=f"h3_1_{i}", bufs=3)
            nc.scalar.activation(tb[:], p[:, B:2 * B], AF.Relu)
            h3s[i] = (ta, tb)
            del h2s[i]

        def emit_l4mm(i):
            if i % 4 == 0:
                p4x[0] = psum.tile([97, B], F32, tag="l4", name=f"p4x_{i // 4}", bufs=2)
            r = 32 * (i % 4)
            for kc in range(2):
                nc.tensor.matmul(
                    p4x[0][r:r + 1, :], lhsT=w4[kc][:], rhs=h3s[i][kc][:],
                    start=(kc == 0), stop=(kc == 1),
                    tile_position=(0, r),
                )
            del h3s[i]

        def emit_drain(g):
            # One exp over the whole [97,512] L4 bank (rows 0/32/64/96 hold
            # steps 4g..4g+3; ACT time scales with free size only, so the
            # stale rows in between cost nothing). Strided DMA scatters the
            # 4 valid rows into score.
            s4 = hpool.tile([97, B], F32, tag="s4", name=f"s4_{g}", bufs=2)
            nc.scalar.activation(s4[:], p4x[0][:], AF.Exp)
            nc.sync.dma_start(out=score[4 * g:4 * g + 4, :], in_=s4[0:97:32, :])

        emit_l1(0)
        emit_l1(1)
        for t in range(RB + 2):
            if t < RB:
                emit_l2mm(t)
                emit_l2epi(t)
            if t >= 1 and t - 1 < RB:
                emit_l3mm(t - 1)
                emit_l3epi(t - 1)
            if t >= 2:
                emit_l4mm(t - 2)
                if (t - 2) % 4 == 3:
                    emit_drain((t - 2) // 4)
            if t + 2 < RB:
                emit_l1(t + 2)

        # ---- tail: softplus = ln(1 + e), one [64, 512] op (ACT time scales
        # with free size only). fp16 output halves the HBM write; two DMAs
        # on different queues overlap the transfer.
        fin = const.tile([RB, B], F16, tag="fin", name="fin")
        nc.scalar.activation(fin[:, :], score[:, :], AF.Ln, bias=1.0)
        nc.sync.dma_start(out=out_d[0:RB // 2, :], in_=fin[0:RB // 2, :])
        nc.gpsimd.dma_start(out=out_d[RB // 2:RB, :], in_=fin[RB // 2:RB, :])


def _emit_generic(tc, nc, d, out_d):
    """Generic path (nonzero biases): original fp32r kernel."""
    AF = mybir.ActivationFunctionType
    OP = mybir.AluOpType
    from contextlib import ExitStack

    with ExitStack() as ctx:
        const = ctx.enter_context(tc.tile_pool(name="const", bufs=1))
        hpool = ctx.enter_context(tc.tile_pool(name="h", bufs=2))
        psum = ctx.enter_context(tc.tile_pool(name="psum", bufs=1, space="PSUM"))

        def load(name, shape, src_ap=None, dt=F32):
            t = const.tile(list(shape), dt, tag=name, name=name + "_s")
            src = src_ap if src_ap is not None else d[name][:]
            if dt == PRE_DT:
                src = src.bitcast(dt)
            nc.sync.dma_start(out=t[:], in_=src)
            return t

        # Load order = first-use order: yb matmuls run first, then xa.
        yT = load("yT", (D, RB), dt=PRE_DT)
        w1b = load("W1b", (D, H), dt=PRE_DT)
        xT = load("xT", (D, B), dt=PRE_DT)
        w1t = load("W1t", (D, H), dt=PRE_DT)
        w2 = [load(f"W2_{k}", (128, H), d["W2"][k * 128:(k + 1) * 128, :], MM_DT) for k in range(2)]
        w3 = [load(f"W3_{k}", (128, H), d["W3"][k * 128:(k + 1) * 128, :], MM_DT) for k in range(2)]
        w4 = [load(f"W4_{k}", (128, 1), d["W4"][k * 128:(k + 1) * 128, :], MM_DT) for k in range(2)]
        b1c = [load(f"b1_{k}", (128, 1), d["b1"][k * 128:(k + 1) * 128, :]) for k in range(2)]
        b2c = [load(f"b2_{k}", (128, 1), d["b2"][k * 128:(k + 1) * 128, :]) for k in range(2)]
        b3c = [load(f"b3_{k}", (128, 1), d["b3"][k * 128:(k + 1) * 128, :]) for k in range(2)]
        b4r = load("b4r", (RB, 1))

        xa = []
        yb = []
        for oc in range(2):
            ms = slice(oc * 128, (oc + 1) * 128)
            pyb = psum.tile([128, RB], F32, tag=f"l3_{oc}", name=f"pyb{oc}", bufs=1)
            nc.tensor.matmul(pyb[:], lhsT=w1b[:, ms], rhs=yT[:], start=True, stop=True)
            ybt = const.tile([128, RB], F32, tag=f"yb{oc}", name=f"yb{oc}")
            nc.vector.tensor_copy(ybt[:], pyb[:])
            yb.append(ybt)
        for oc in range(2):
            ms = slice(oc * 128, (oc + 1) * 128)
            pxa = psum.tile([128, B], F32, tag=f"l2_{oc}", name=f"pxa{oc}", bufs=2)
            nc.tensor.matmul(pxa[:], lhsT=w1t[:, ms], rhs=xT[:], start=True, stop=True)
            xat = const.tile([128, B], F32, tag=f"xa{oc}", name=f"xa{oc}")
            nc.scalar.activation(xat[:], pxa[:], AF.Identity, bias=b1c[oc][:, 0:1])
            xa.append(xat)

        score = const.tile([RB, B], F32, tag="score", name="score")

        h1s, h2s, h3s, p4s = {}, {}, {}, {}

        def emit_l1(i):
            for oc in range(2):
                t = hpool.tile([128, B], MM_DT, tag=f"h1_{oc}", name=f"h1_{oc}_{i}", bufs=4)
                nc.vector.tensor_scalar(
                    t[:], xa[oc][:], yb[oc][:, i:i + 1], 0.0, OP.add, OP.max
                )
                h1s[i, oc] = t

        def emit_l2mm(i):
            for mc in range(2):
                ms = slice(mc * 128, (mc + 1) * 128)
                p = psum.tile([128, B], F32, tag=f"l2_{mc}", name=f"p2_{mc}_{i}", bufs=2)
                for kc in range(2):
                    nc.tensor.matmul(
                        p[:], lhsT=w2[kc][:, ms], rhs=h1s[i, kc][:],
                        start=(kc == 0), stop=(kc == 1),
                    )
                h2s[i, mc] = p

        def emit_l2epi(i):
            for mc in range(2):
                p = h2s[i, mc]
                t = hpool.tile([128, B], MM_DT, tag=f"h2_{mc}", name=f"h2_{mc}_{i}", bufs=3)
                nc.scalar.activation(t[:], p[:], AF.Relu, bias=b2c[mc][:, 0:1])
                h2s[i, mc] = t

        def emit_l3mm(i):
            for mc in range(2):
                ms = slice(mc * 128, (mc + 1) * 128)
                p = psum.tile([128, B], F32, tag=f"l3_{mc}", name=f"p3_{mc}_{i}", bufs=1)
                for kc in range(2):
                    nc.tensor.matmul(
                        p[:], lhsT=w3[kc][:, ms], rhs=h2s[i, kc][:],
                        start=(kc == 0), stop=(kc == 1),
                    )
                h3s[i, mc] = p

        def emit_l3epi(i):
            for mc in range(2):
                p = h3s[i, mc]
                t = hpool.tile([128, B], MM_DT, tag=f"h3_{mc}", name=f"h3_{mc}_{i}", bufs=3)
                nc.vector.tensor_scalar(t[:], p[:], b3c[mc][:, 0:1], 0.0, OP.add, OP.max)
                h3s[i, mc] = t
            del h2s[i, 0], h2s[i, 1]

        def emit_l4mm(i):
            p4 = psum.tile([1, B], F32, tag="l4", name=f"p4_{i}", bufs=2)
            for kc in range(2):
                nc.tensor.matmul(
                    p4[:], lhsT=w4[kc][:], rhs=h3s[i, kc][:],
                    start=(kc == 0), stop=(kc == 1),
                )
            p4s[i] = p4
            del h3s[i, 0], h3s[i, 1]

        def emit_drain(i):
            s4 = hpool.tile([1, B], F32, tag="s4", name=f"s4_{i}", bufs=4)
            nc.scalar.activation(s4[:], p4s.pop(i)[:], AF.Exp, bias=b4r[0:1, 0:1])
            nc.sync.dma_start(out=score[i:i + 1, :], in_=s4[:])

        fin = const.tile([RB, B], F32, tag="fin", name="fin")

        def emit_tail(lo, hi):
            nc.scalar.activation(fin[lo:hi, :], score[lo:hi, :], AF.Ln, bias=1.0)
            nc.sync.dma_start(out=out_d[lo:hi, :], in_=fin[lo:hi, :])

        HALF = RB // 2
        emit_l1(0)
        for t in range(RB + 2):
            if t + 1 < RB:
                emit_l1(t + 1)
            if t < RB:
                emit_l2mm(t)
                emit_l2epi(t)
            if t >= 2:
                emit_l4mm(t - 2)
                emit_drain(t - 2)
            if t >= 1 and t - 1 < RB:
                emit_l3mm(t - 1)
                emit_l3epi(t - 1)
            if t - 2 == HALF - 1:
                emit_tail(0, HALF)

        emit_tail(HALF, RB)


def _build_program(fast):
    nc = bacc.Bacc("TRN2", target_bir_lowering=False, debug=False, enable_asserts=False)
    d = {}
    in_dt = F16 if fast else F32
    specs = [
        ("xT", (D, B)), ("yT", (D, RB)),
        ("W1t", (D, H)), ("W1b", (D, H)),
        ("W2", (H, H)), ("W3", (H, H)), ("W4", (H, 1)),
    ]
    if not fast:
        specs += [
            ("b1", (H, 1)), ("b2", (H, 1)), ("b3", (H, 1)),
            ("b4r", (RB, 1)),
        ]
    for name, shape in specs:
        d[name] = nc.dram_tensor(name, list(shape), in_dt, kind="ExternalInput").ap()
    # fp16 output on the fast path: halves the HBM write; host converts back.
    out_d = nc.dram_tensor("out", [RB, B], F16 if fast else F32, kind="ExternalOutput").ap()
    with tile.TileContext(nc) as tc:
        if fast:
            _emit_f16(tc, nc, d, out_d)
        else:
            _emit_generic(tc, nc, d, out_d)
    nc.compile()
    return nc


_PROGRAMS = {}


def _get_program(fast):
    if fast not in _PROGRAMS:
        _PROGRAMS[fast] = _build_program(fast)
    return _PROGRAMS[fast]


def _make_in_maps(fast, x, y, W1, b1, W2, b2, W3, b3, W4, b4):
    f = np.float16 if fast else np.float32
    shared = {
        "xT": np.ascontiguousarray(x.T, dtype=f),
        "W1t": np.ascontiguousarray(W1[:D], dtype=f),
        "W1b": np.ascontiguousarray(W1[D:], dtype=f),
        "W2": np.ascontiguousarray(W2, dtype=f),
        "W3": np.ascontiguousarray(W3, dtype=f),
        "W4": np.ascontiguousarray(W4.reshape(H, 1), dtype=f),
    }
    if not fast:
        f32 = np.float32
        shared.update({
            "b1": np.ascontiguousarray(b1.reshape(H, 1), dtype=f32),
            "b2": np.ascontiguousarray(b2.reshape(H, 1), dtype=f32),
            "b3": np.ascontiguousarray(b3.reshape(H, 1), dtype=f32),
            "b4r": np.full((RB, 1), np.asarray(b4, dtype=f32).reshape(-1)[0], dtype=f32),
        })
    in_maps = []
    for c in range(NCORES):
        m = dict(shared)
        m["yT"] = np.ascontiguousarray(y[c * RB:(c + 1) * RB].T, dtype=f)
        in_maps.append(m)
    return in_maps


def _run(inputs, trace=False, trace_cores=None):
    fast = all(
        float(np.abs(np.asarray(inputs[b], dtype=np.float32)).max()) == 0.0
        for b in ("b1", "b2", "b3", "b4")
    )
    nc = _get_program(fast)
    in_maps = _make_in_maps(fast, **inputs)
    res = run_bass_kernel_spmd(
        nc, in_maps, list(range(NCORES)), trace=trace, trace_cores=trace_cores,
    )
    V = np.concatenate([res.results[c]["out"] for c in range(NCORES)], axis=0)
    out = np.ascontiguousarray(V.T, dtype=np.float32)
    return out, res


def kernel(**inputs):
    out, _ = _run(inputs, trace=False)
    return out


# revision 9
# speedup vs baseline: 1.0035x; 1.0035x over previous
"""ConcatCritic MLP over the B^2 pair grid, Trainium2 Bass/Tile kernel.

out[i, j] = softplus(f(x[i], y[j])) where f is a 4-layer MLP on
concat(x, y). Reference pair grid: pairs[a*n+b] = concat(x[b], y[a]),
scores.reshape(n,n).T -> out.

Key factorization: layer 1 is linear in the concat, so
  h1[a,b] = relu(x[b] @ W1top + y[a] @ W1bot + b1)
with W1top = W1[:128], W1bot = W1[128:]. The [B^2, 256] layer-1 matmul
collapses into two tiny matmuls plus a per-partition broadcast add.

Layout: activations kept transposed [features, batch] so every layer's
matmul (weights stationary as lhsT [K, M]) writes the next layer's rhs
directly: out[m=feat, n=j] = sum_k W[k, m] * hT[k, j].

Sharding: core c owns y rows [c*64, (c+1)*64); computes block
V_c[il, j] = f(x[j], y[c*64+il]) of shape [64, 512]. Host gathers
V = concat(V_c) and returns V.T.

Fast path (all biases zero, which is what setup_inputs produces):
fp16 weights + activations. PE matmul stays 1 cycle/row (same as
fp32r) but LDWEIGHTS halves, DVE L1 runs in 4x perf mode, input DMA
halves, and the drain path avoids the mid-run Ln that forced act-table
reloads in the fp32r variant (4 loads -> 2). Measured fp16 rel err vs
fp64 reference ~4e-4 (budget 2e-2).

Generic path (any nonzero bias): the original fp32r kernel, unchanged.

Self-contained: hardcodes shapes; imports concourse from the system repo.
"""

import os
import sys

import numpy as np
import ml_dtypes


def _import_concourse():
    try:
        import concourse  # noqa: F401
        return
    except ImportError:
        pass
    for p in ("/opt/trn_rl_repo", "/root/.axon_site/_ro/trn_rl_repo"):
        if os.path.isdir(p) and p not in sys.path:
            sys.path.insert(0, p)
    import concourse  # noqa: F401


_import_concourse()

import concourse.bacc as bacc  # noqa: E402
import concourse.tile as tile  # noqa: E402
from concourse import mybir  # noqa: E402
from concourse.bass_utils import run_bass_kernel_spmd  # noqa: E402

B = 512          # batch (pair-grid side)
D = 128          # input dim per tensor
H = 256          # hidden dim
NCORES = 8
RB = B // NCORES  # 64 y-rows per core
F32 = mybir.dt.float32
F16 = mybir.dt.float16

# float32r: fp32 bits in memory, single-pass reduced-precision multiply on
# the PE at 1 cycle/row (vs 4 for full fp32). Used by the generic path.
PRE_DT = mybir.dt.float32r
MM_DT = mybir.dt.float32r


def _emit_f16(tc, nc, d, out_d):
    """Fast path: all biases zero, fp16 weights/activations.

    Per step the PE runs 10 matmuls (~216ns each incl. hidden fp16
    LDWEIGHTS) in order L2MM(t) | L3MM(t-1) | L4MM(t-2), so every PE
    instruction depends only on epilogue work from a previous step.
    ACT: one merged [128,1024] relu over both L2 psum banks + L3epi mc1
    (~1.8us). DVE: L3epi mc0 early, then 2x L1 in 4x perf mode
    (~1.4us). exp() batches 4 steps via one [97,512] pass over the
    shared L4 psum bank (matmuls write partition bases 0/32/64/96);
    a strided DMA scatters the 4 valid rows into score. Single Ln at
    the end: exactly two act-table loads for the whole kernel.
    PSUM banks: l2 [128,1024] bufs=2 (4) + l3 [128,1024] bufs=1 (2) +
    l4 [97,512] bufs=2 (2) = 8 exactly.
    """
    AF = mybir.ActivationFunctionType
    OP = mybir.AluOpType
    from contextlib import ExitStack

    with ExitStack() as ctx:
        const = ctx.enter_context(tc.tile_pool(name="const", bufs=1))
        hpool = ctx.enter_context(tc.tile_pool(name="h", bufs=2))
        psum = ctx.enter_context(tc.tile_pool(name="psum", bufs=1, space="PSUM"))

        def load(name, shape, src_ap=None, eng=None):
            t = const.tile(list(shape), F16, tag=name, name=name + "_s")
            src = src_ap if src_ap is not None else d[name][:]
            (eng or nc.sync).dma_start(out=t[:], in_=src)
            return t

        # Load order = first-use order. The per-DMA trigger costs ~600ns of
        # engine-queue time, so split issue across the idle gpsimd queue
        # (first-needed tensors) and sync (weights needed a few steps in).
        yT = load("yT", (D, RB), eng=nc.gpsimd)
        w1b = load("W1b", (D, H), eng=nc.gpsimd)
        xT = load("xT", (D, B), eng=nc.gpsimd)
        w1t = load("W1t", (D, H), eng=nc.gpsimd)
        w2 = [load(f"W2_{k}", (128, H), d["W2"][k * 128:(k + 1) * 128, :]) for k in range(2)]
        w3 = [load(f"W3_{k}", (128, H), d["W3"][k * 128:(k + 1) * 128, :]) for k in range(2)]
        w4 = [load(f"W4_{k}", (128, 1), d["W4"][k * 128:(k + 1) * 128, :]) for k in range(2)]

        # ---- preamble: xa[oc] = (x @ W1top)^T, yb[oc] = (y_slice @ W1bot)^T
        # Preamble psum tiles borrow the main-loop l2/l3 tags (no extra
        # banks). yb first (tiny copies), xa epilogue on ACT so the DVE can
        # start L1(0) as soon as xa lands.
        yb = []
        pl3 = psum.tile([128, 2 * B], F32, tag="l3", name="pyb", bufs=1)
        for oc in range(2):
            ms = slice(oc * 128, (oc + 1) * 128)
            nc.tensor.matmul(pl3[:, oc * B:oc * B + RB], lhsT=w1b[:, ms], rhs=yT[:],
                             start=True, stop=True)
            ybt = const.tile([128, RB], F32, tag=f"yb{oc}", name=f"yb{oc}")
            nc.vector.tensor_copy(ybt[:], pl3[:, oc * B:oc * B + RB])
            yb.append(ybt)
        xa = []
        pl2 = psum.tile([128, 2 * B], F32, tag="l2", name="pxa", bufs=2)
        for oc in range(2):
            ms = slice(oc * 128, (oc + 1) * 128)
            nc.tensor.matmul(pl2[:, oc * B:(oc + 1) * B], lhsT=w1t[:, ms], rhs=xT[:],
                             start=True, stop=True)
            xat = const.tile([128, B], F16, tag=f"xa{oc}", name=f"xa{oc}")
            nc.scalar.activation(xat[:], pl2[:, oc * B:(oc + 1) * B], AF.Copy, bias=0.0)
            xa.append(xat)

        score = const.tile([RB, B], F32, tag="score", name="score")

        h1s, h2s, h3s = {}, {}, {}
        p4x = [None]  # current 4-step L4 psum tile

        def emit_l1(i):
            # DVE, 4x perf mode: fp16 in/out, per-partition fp32 scalar yb.
            for oc in range(2):
                t = hpool.tile([128, B], F16, tag=f"h1_{oc}", name=f"h1_{oc}_{i}", bufs=5)
                nc.vector.tensor_scalar(
                    t[:], xa[oc][:], yb[oc][:, i:i + 1], 0.0, OP.add, OP.max
                )
                h1s[i, oc] = t

        def emit_l2mm(i):
            p = psum.tile([128, 2 * B], F32, tag="l2", name=f"p2_{i}", bufs=2)
            for mc in range(2):
                ms = slice(mc * 128, (mc + 1) * 128)
                for kc in range(2):
                    nc.tensor.matmul(
                        p[:, mc * B:(mc + 1) * B], lhsT=w2[kc][:, ms], rhs=h1s[i, kc][:],
                        start=(kc == 0), stop=(kc == 1),
                    )
            h2s[i] = p

        def emit_l2epi(i):
            # One merged [128,1024] relu over both L2 psum banks (ACT).
            p = h2s[i]
            t = hpool.tile([128, 2 * B], F16, tag="h2", name=f"h2_{i}", bufs=4)
            nc.scalar.activation(t[:], p[:], AF.Relu)
            h2s[i] = t

        def emit_l3mm(i):
            p = psum.tile([128, 2 * B], F32, tag="l3", name=f"p3_{i}", bufs=1)
            for mc in range(2):
                ms = slice(mc * 128, (mc + 1) * 128)
                for kc in range(2):
                    nc.tensor.matmul(
                        p[:, mc * B:(mc + 1) * B], lhsT=w3[kc][:, ms],
                        rhs=h2s[i][:, kc * B:(kc + 1) * B],
                        start=(kc == 0), stop=(kc == 1),
                    )
            h3s[i] = p

        def emit_l3epi(i):
            # mc0 on DVE, mc1 on ACT: balances both engines under the PE
            # step time (ACT ~1.96us, DVE ~1.4us vs 2.16us PE).
            p = h3s[i]
            ta = hpool.tile([128, B], F16, tag="h3_0", name=f"h3_0_{i}", bufs=4)
            nc.vector.tensor_scalar_max(ta[:], p[:, 0:B], 0.0)
            tb = hpool.tile([128, B], F16, tag="h3_1", name=f"h3_1_{i}", bufs=4)
            nc.scalar.activation(tb[:], p[:, B:2 * B], AF.Relu)
            h3s[i] = (ta, tb)
            del h2s[i]

        def emit_l4mm(i):
            if i % 4 == 0:
                p4x[0] = psum.tile([97, B], F32, tag="l4", name=f"p4x_{i // 4}", bufs=2)
            r = 32 * (i % 4)
            for kc in range(2):
                nc.tensor.matmul(
                    p4x[0][r:r + 1, :], lhsT=w4[kc][:], rhs=h3s[i][kc][:],
                    start=(kc == 0), stop=(kc == 1),
                    tile_position=(0, r),
                )
            del h3s[i]

        def emit_drain(g):
            # One exp over the whole [97,512] L4 bank (rows 0/32/64/96 hold
            # steps 4g..4g+3; ACT time scales with free size only, so the
            # stale rows in between cost nothing). Strided DMA scatters the
            # 4 valid rows into score.
            s4 = hpool.tile([97, B], F32, tag="s4", name=f"s4_{g}", bufs=2)
            nc.scalar.activation(s4[:], p4x[0][:], AF.Exp)
            nc.sync.dma_start(out=score[4 * g:4 * g + 4, :], in_=s4[0:97:32, :])

        # All PE consumers lag their producers' emission by >= 2 iterations:
        # per iter the PE runs L2MM(t) | L3MM(t-2) | L4MM(t-4), so every
        # epilogue has a full step of slack before its result is needed and
        # engine-queue jitter never stalls the PE.
        emit_l1(0)
        emit_l1(1)
        for t in range(RB + 4):
            if t < RB:
                emit_l2mm(t)
                emit_l2epi(t)
            if 2 <= t < RB + 2:
                emit_l3mm(t - 2)
                emit_l3epi(t - 2)
            if 4 <= t < RB + 4:
                emit_l4mm(t - 4)
                if (t - 4) % 4 == 3:
                    emit_drain((t - 4) // 4)
            if t + 2 < RB:
                emit_l1(t + 2)

        # ---- tail: softplus = ln(1 + e), one [64, 512] op (ACT time scales
        # with free size only). fp16 output halves the HBM write; two DMAs
        # on different queues overlap the transfer.
        fin = const.tile([RB, B], F16, tag="fin", name="fin")
        nc.scalar.activation(fin[:, :], score[:, :], AF.Ln, bias=1.0)
        nc.sync.dma_start(out=out_d[0:RB // 2, :], in_=fin[0:RB // 2, :])
        nc.gpsimd.dma_start(out=out_d[RB // 2:RB, :], in_=fin[RB // 2:RB, :])


def _emit_generic(tc, nc, d, out_d):
    """Generic path (nonzero biases): original fp32r kernel."""
    AF = mybir.ActivationFunctionType
    OP = mybir.AluOpType
    from contextlib import ExitStack

    with ExitStack() as ctx:
        const = ctx.enter_context(tc.tile_pool(name="const", bufs=1))
        hpool = ctx.enter_context(tc.tile_pool(name="h", bufs=2))
        psum = ctx.enter_context(tc.tile_pool(name="psum", bufs=1, space="PSUM"))

        def load(name, shape, src_ap=None, dt=F32):
            t = const.tile(list(shape), dt, tag=name, name=name + "_s")
            src = src_ap if src_ap is not None else d[name][:]
            if dt == PRE_DT:
                src = src.bitcast(dt)
            nc.sync.dma_start(out=t[:], in_=src)
            return t

        # Load order = first-use order: yb matmuls run first, then xa.
        yT = load("yT", (D, RB), dt=PRE_DT)
        w1b = load("W1b", (D, H), dt=PRE_DT)
        xT = load("xT", (D, B), dt=PRE_DT)
        w1t = load("W1t", (D, H), dt=PRE_DT)
        w2 = [load(f"W2_{k}", (128, H), d["W2"][k * 128:(k + 1) * 128, :], MM_DT) for k in range(2)]
        w3 = [load(f"W3_{k}", (128, H), d["W3"][k * 128:(k + 1) * 128, :], MM_DT) for k in range(2)]
        w4 = [load(f"W4_{k}", (128, 1), d["W4"][k * 128:(k + 1) * 128, :], MM_DT) for k in range(2)]
        b1c = [load(f"b1_{k}", (128, 1), d["b1"][k * 128:(k + 1) * 128, :]) for k in range(2)]
        b2c = [load(f"b2_{k}", (128, 1), d["b2"][k * 128:(k + 1) * 128, :]) for k in range(2)]
        b3c = [load(f"b3_{k}", (128, 1), d["b3"][k * 128:(k + 1) * 128, :]) for k in range(2)]
        b4r = load("b4r", (RB, 1))

        xa = []
        yb = []
        for oc in range(2):
            ms = slice(oc * 128, (oc + 1) * 128)
            pyb = psum.tile([128, RB], F32, tag=f"l3_{oc}", name=f"pyb{oc}", bufs=1)
            nc.tensor.matmul(pyb[:], lhsT=w1b[:, ms], rhs=yT[:], start=True, stop=True)
            ybt = const.tile([128, RB], F32, tag=f"yb{oc}", name=f"yb{oc}")
            nc.vector.tensor_copy(ybt[:], pyb[:])
            yb.append(ybt)
        for oc in range(2):
            ms = slice(oc * 128, (oc + 1) * 128)
            pxa = psum.tile([128, B], F32, tag=f"l2_{oc}", name=f"pxa{oc}", bufs=2)
            nc.tensor.matmul(pxa[:], lhsT=w1t[:, ms], rhs=xT[:], start=True, stop=True)
            xat = const.tile([128, B], F32, tag=f"xa{oc}", name=f"xa{oc}")
            nc.scalar.activation(xat[:], pxa[:], AF.Identity, bias=b1c[oc][:, 0:1])
            xa.append(xat)

        score = const.tile([RB, B], F32, tag="score", name="score")

        h1s, h2s, h3s, p4s = {}, {}, {}, {}

        def emit_l1(i):
            for oc in range(2):
                t = hpool.tile([128, B], MM_DT, tag=f"h1_{oc}", name=f"h1_{oc}_{i}", bufs=4)
                nc.vector.tensor_scalar(
                    t[:], xa[oc][:], yb[oc][:, i:i + 1], 0.0, OP.add, OP.max
                )
                h1s[i, oc] = t

        def emit_l2mm(i):
            for mc in range(2):
                ms = slice(mc * 128, (mc + 1) * 128)
                p = psum.tile([128, B], F32, tag=f"l2_{mc}", name=f"p2_{mc}_{i}", bufs=2)
                for kc in range(2):
                    nc.tensor.matmul(
                        p[:], lhsT=w2[kc][:, ms], rhs=h1s[i, kc][:],
                        start=(kc == 0), stop=(kc == 1),
                    )
                h2s[i, mc] = p

        def emit_l2epi(i):
            for mc in range(2):
                p = h2s[i, mc]
                t = hpool.tile([128, B], MM_DT, tag=f"h2_{mc}", name=f"h2_{mc}_{i}", bufs=3)
                nc.scalar.activation(t[:], p[:], AF.Relu, bias=b2c[mc][:, 0:1])
                h2s[i, mc] = t

        def emit_l3mm(i):
            for mc in range(2):
                ms = slice(mc * 128, (mc + 1) * 128)
                p = psum.tile([128, B], F32, tag=f"l3_{mc}", name=f"p3_{mc}_{i}", bufs=1)
                for kc in range(2):
                    nc.tensor.matmul(
                        p[:], lhsT=w3[kc][:, ms], rhs=h2s[i, kc][:],
                        start=(kc == 0), stop=(kc == 1),
                    )
                h3s[i, mc] = p

        def emit_l3epi(i):
            for mc in range(2):
                p = h3s[i, mc]
                t = hpool.tile([128, B], MM_DT, tag=f"h3_{mc}", name=f"h3_{mc}_{i}", bufs=3)
                nc.vector.tensor_scalar(t[:], p[:], b3c[mc][:, 0:1], 0.0, OP.add, OP.max)
                h3s[i, mc] = t
            del h2s[i, 0], h2s[i, 1]

        def emit_l4mm(i):
            p4 = psum.tile([1, B], F32, tag="l4", name=f"p4_{i}", bufs=2)
            for kc in range(2):
                nc.tensor.matmul(
                    p4[:], lhsT=w4[kc][:], rhs=h3s[i, kc][:],
                    start=(kc == 0), stop=(kc == 1),
                )
            p4s[i] = p4
            del h3s[i, 0], h3s[i, 1]

        def emit_drain(i):
            s4 = hpool.tile([1, B], F32, tag="s4", name=f"s4_{i}", bufs=4)
            nc.scalar.activation(s4[:], p4s.pop(i)[:], AF.Exp, bias=b4r[0:1, 0:1])
            nc.sync.dma_start(out=score[i:i + 1, :], in_=s4[:])

        fin = const.tile([RB, B], F32, tag="fin", name="fin")

        def emit_tail(lo, hi):
            nc.scalar.activation(fin[lo:hi, :], score[lo:hi, :], AF.Ln, bias=1.0)
            nc.sync.dma_start(out=out_d[lo:hi, :], in_=fin[lo:hi, :])

        HALF = RB // 2
        emit_l1(0)
        for t in range(RB + 2):
            if t + 1 < RB:
                emit_l1(t + 1)
            if t < RB:
                emit_l2mm(t)
                emit_l2epi(t)
            if t >= 2:
                emit_l4mm(t - 2)
                emit_drain(t - 2)
            if t >= 1 and t - 1 < RB:
                emit_l3mm(t - 1)
                emit_l3epi(t - 1)
            if t - 2 == HALF - 1:
                emit_tail(0, HALF)

        emit_tail(HALF, RB)


def _build_program(fast):
    nc = bacc.Bacc("TRN2", target_bir_lowering=False, debug=False, enable_asserts=False)
    d = {}
    in_dt = F16 if fast else F32
    specs = [
        ("xT", (D, B)), ("yT", (D, RB)),
        ("W1t", (D, H)), ("W1b", (D, H)),
        ("W2", (H, H)), ("W3", (H, H)), ("W4", (H, 1)),
    ]
    if not fast:
        specs += [
            ("b1", (H, 1)), ("b2", (H, 1)), ("b3", (H, 1)),
            ("b4r", (RB, 1)),
        ]
    for name, shape in specs:
        d[name] = nc.dram_tensor(name, list(shape), in_dt, kind="ExternalInput").ap()
    # fp16 output on the fast path: halves the HBM write; host converts back.
    out_d = nc.dram_tensor("out", [RB, B], F16 if fast else F32, kind="ExternalOutput").ap()
    with tile.TileContext(nc) as tc:
        if fast:
            _emit_f16(tc, nc, d, out_d)
        else:
            _emit_generic(tc, nc, d, out_d)
    nc.compile()
    return nc


_PROGRAMS = {}


def _get_program(fast):
    if fast not in _PROGRAMS:
        _PROGRAMS[fast] = _build_program(fast)
    return _PROGRAMS[fast]


def _make_in_maps(fast, x, y, W1, b1, W2, b2, W3, b3, W4, b4):
    f = np.float16 if fast else np.float32
    shared = {
        "xT": np.ascontiguousarray(x.T, dtype=f),
        "W1t": np.ascontiguousarray(W1[:D], dtype=f),
        "W1b": np.ascontiguousarray(W1[D:], dtype=f),
        "W2": np.ascontiguousarray(W2, dtype=f),
        "W3": np.ascontiguousarray(W3, dtype=f),
        "W4": np.ascontiguousarray(W4.reshape(H, 1), dtype=f),
    }
    if not fast:
        f32 = np.float32
        shared.update({
            "b1": np.ascontiguousarray(b1.reshape(H, 1), dtype=f32),
            "b2": np.ascontiguousarray(b2.reshape(H, 1), dtype=f32),
            "b3": np.ascontiguousarray(b3.reshape(H, 1), dtype=f32),
            "b4r": np.full((RB, 1), np.asarray(b4, dtype=f32).reshape(-1)[0], dtype=f32),
        })
    in_maps = []
    for c in range(NCORES):
        m = dict(shared)
        m["yT"] = np.ascontiguousarray(y[c * RB:(c + 1) * RB].T, dtype=f)
        in_maps.append(m)
    return in_maps


def _run(inputs, trace=False, trace_cores=None):
    fast = all(
        float(np.abs(np.asarray(inputs[b], dtype=np.float32)).max()) == 0.0
        for b in ("b1", "b2", "b3", "b4")
    )
    nc = _get_program(fast)
    in_maps = _make_in_maps(fast, **inputs)
    res = run_bass_kernel_spmd(
        nc, in_maps, list(range(NCORES)), trace=trace, trace_cores=trace_cores,
    )
    V = np.concatenate([res.results[c]["out"] for c in range(NCORES)], axis=0)
    out = np.ascontiguousarray(V.T, dtype=np.float32)
    return out, res


def kernel(**inputs):
    out, _ = _run(inputs, trace=False)
    return out
